# revision 6
# baseline (speedup 1.0000x reference)
"""Trainium2 Bass kernel for AttentionConvFull (local 5x5 window attention
with per-channel softmax, grouped 1x1 conv projections).

Sharding: 8 cores = batch(4) x H-halves(2). Each core gets a 32-row halo'd,
zero-padded slice of x, pre-transposed on host to channel-major [256, 32*60].
No collectives needed.

Per-core dataflow (2 channel-chunks of 128 partitions each):
  PE    : block-diag 128x128 fp32 matmuls for q/k/v projections; per window
          offset j, identity-matmul PSUM accumulation of den += e_j and
          num += (e_j * v_j)  (bf16 inputs, fp32 accumulate)
  DMA   : creates 5 column-shifted flat copies of the k/v maps (SBUF->SBUF)
          so every j-loop operand is a flat, 4B-aligned [128, N] slice
  GpSimd: kr = k_dj + rel_j (tensor_scalar, per-partition scalar)
  DVE   : t = kr * q, w = e * v_dj (flat bf16 tensor_tensor, 2x mode)
  ACT   : e = exp(t); projection PSUM->SBUF casts (+q_emb bias for q)
  Epilogue: out = num * recip_approx(den), DMA out channel-major; host
  reassembles to (B,H,W,C).
"""

import numpy as np
import ml_dtypes

import concourse.bass as bass
import concourse.tile as tile
from concourse import bacc, mybir
from concourse.bass_utils import run_bass_kernel_spmd

F32 = mybir.dt.float32
BF16 = mybir.dt.bfloat16

K = 5
G = 8
B, H, W, C = 4, 56, 56, 256
Cg = C // G            # 32
P = K // 2             # 2
HS = H // 2            # 28 output rows per shard
MR = HS + 2 * P        # 32 map rows
MC = W + 2 * P         # 60 map cols
SP = MR * MC           # 1920 map spatial
OP = HS * W            # 1568 output spatial per shard
FM = MR * W            # 1792 flat shifted-map size
NCH = 2                # channel chunks of 128 partitions
NCORES = 8
HALF = OP // 2         # 784: PSUM accumulate tile half-size

# which engine does the kr = k + rel_j add: "gpsimd" or "vector"
RADD_ENGINE = "gpsimd"


def build_nc():
    nc = bacc.Bacc(
        "TRN2", target_bir_lowering=False, debug=False, num_devices=NCORES
    )

    xt_d = nc.dram_tensor("xt", [NCH, 128, SP], F32, kind="ExternalInput").ap()
    wq_d = nc.dram_tensor("wqb", [NCH, 128, 128], F32, kind="ExternalInput").ap()
    wk_d = nc.dram_tensor("wkb", [NCH, 128, 128], F32, kind="ExternalInput").ap()
    wv_d = nc.dram_tensor("wvb", [NCH, 128, 128], F32, kind="ExternalInput").ap()
    rel_d = nc.dram_tensor("relb", [NCH, 128, K * K], F32, kind="ExternalInput").ap()
    qe_d = nc.dram_tensor("qeb", [NCH, 128, 1], F32, kind="ExternalInput").ap()
    id_d = nc.dram_tensor("idn", [128, 128], BF16, kind="ExternalInput").ap()
    out_d = nc.dram_tensor("out", [NCH, 128, OP], F32, kind="ExternalOutput").ap()

    with tile.TileContext(nc) as tc:
        with (
            tc.tile_pool(name="consts", bufs=1) as consts,
            tc.tile_pool(name="weights", bufs=2) as wpool,
            tc.tile_pool(name="xin", bufs=2) as xpool,
            tc.tile_pool(name="maps", bufs=2) as mpool,
            tc.tile_pool(name="jwork", bufs=4) as jpool,
            tc.tile_pool(name="epi", bufs=2) as epool,
            tc.tile_pool(name="acc", bufs=4, space=bass.MemorySpace.PSUM) as psum,
        ):
            ident = consts.tile([128, 128], BF16, tag="ident")
            nc.sync.dma_start(ident[:], id_d)

            # ---- per-chunk persistent maps ----
            kvars, vvars, qflats, rels = [], [], [], []

            for c in range(NCH):
                x_sb = xpool.tile([128, SP], F32, tag="x")
                nc.sync.dma_start(x_sb[:], xt_d[c])

                wts = {}
                for nm, d in (("wq", wq_d), ("wk", wk_d), ("wv", wv_d)):
                    t = wpool.tile([128, 128], F32, tag=nm)
                    nc.sync.dma_start(t[:], d[c])
                    wts[nm] = t
                rel_sb = wpool.tile([128, K * K], F32, tag="rel")
                nc.sync.dma_start(rel_sb[:], rel_d[c])
                qe_sb = wpool.tile([128, 1], F32, tag="qe")
                nc.sync.dma_start(qe_sb[:], qe_d[c])
                rels.append(rel_sb)

                # base maps (bf16), psum -> sbuf casts on ACT
                k_bf = mpool.tile([128, SP], BF16, tag="k")
                v_bf = mpool.tile([128, SP], BF16, tag="v")
                q_bf = mpool.tile([128, SP], BF16, tag="q")

                # projections: 4 slices of 480 cols each
                NS = 4
                SL = SP // NS  # 480
                for s in range(NS):
                    lo = s * SL
                    rhs = x_sb[:, lo : lo + SL]
                    for nm in ("wk", "wv", "wq"):
                        ps = psum.tile([128, HALF], F32, tag="acc", name=f"pp{c}{s}{nm}")
                        nc.tensor.matmul(
                            ps[:, :SL], wts[nm][:], rhs, start=True, stop=True
                        )
                        if nm == "wq":
                            # q = proj + q_emb (per-partition bias), cast bf16
                            nc.scalar.activation(
                                q_bf[:, lo : lo + SL],
                                ps[:, :SL],
                                mybir.ActivationFunctionType.Identity,
                                bias=qe_sb[:],
                            )
                        elif nm == "wk":
                            nc.scalar.copy(k_bf[:, lo : lo + SL], ps[:, :SL])
                        else:
                            nc.scalar.copy(v_bf[:, lo : lo + SL], ps[:, :SL])

                # flat dj-shifted variants via SBUF->SBUF DMA (free engines):
                # k_dj[h*56+w] = k[h, w+dj], shape [128, 32*56]
                k3 = k_bf[:].rearrange("p (h w) -> p h w", h=MR)
                v3 = v_bf[:].rearrange("p (h w) -> p h w", h=MR)
                kvs, vvs = [], []
                for dj in range(K):
                    kd = mpool.tile([128, FM], BF16, tag=f"kd{dj}", name=f"kd{c}{dj}")
                    nc.sync.dma_start(
                        kd[:].rearrange("p (h w) -> p h w", h=MR),
                        k3[:, :, dj : dj + W],
                    )
                    kvs.append(kd)
                    vd = mpool.tile([128, FM], BF16, tag=f"vd{dj}", name=f"vd{c}{dj}")
                    nc.sync.dma_start(
                        vd[:].rearrange("p (h w) -> p h w", h=MR),
                        v3[:, :, dj : dj + W],
                    )
                    vvs.append(vd)
                kvars.append(kvs)
                vvars.append(vvs)

                qf = mpool.tile([128, OP], BF16, tag="qf", name=f"qf{c}")
                q3 = q_bf[:].rearrange("p (h w) -> p h w", h=MR)
                nc.sync.dma_start(
                    qf[:].rearrange("p (h w) -> p h w", h=HS),
                    q3[:, P : P + HS, P : P + W],
                )
                qflats.append(qf)

            # ---- j-loop per chunk ----
            for c in range(NCH):
                rel_sb, qf = rels[c], qflats[c]

                den = [
                    psum.tile([128, HALF], F32, tag="acc", name=f"den{c}{h}")
                    for h in range(2)
                ]
                num = [
                    psum.tile([128, HALF], F32, tag="acc", name=f"num{c}{h}")
                    for h in range(2)
                ]

                for j in range(K * K):
                    di, dj = divmod(j, K)
                    o = di * W
                    kv = kvars[c][dj][:, o : o + OP]
                    vv = vvars[c][dj][:, o : o + OP]

                    kr_t = jpool.tile([128, OP], BF16, tag="kr", name=f"kr{c}{j}")
                    radd = nc.gpsimd if RADD_ENGINE == "gpsimd" else nc.vector
                    radd.tensor_scalar(
                        kr_t[:],
                        kv,
                        rel_sb[:, j : j + 1],
                        None,
                        mybir.AluOpType.add,
                    )

                    t_t = jpool.tile([128, OP], BF16, tag="t", name=f"t{c}{j}")
                    nc.vector.tensor_tensor(
                        t_t[:], kr_t[:], qf[:], mybir.AluOpType.mult
                    )

                    e_t = jpool.tile([128, OP], BF16, tag="e", name=f"e{c}{j}")
                    nc.scalar.activation(
                        e_t[:], t_t[:], mybir.ActivationFunctionType.Exp
                    )

                    w_t = jpool.tile([128, OP], BF16, tag="w", name=f"w{c}{j}")
                    nc.vector.tensor_tensor(
                        w_t[:], e_t[:], vv, mybir.AluOpType.mult
                    )

                    st = j == 0
                    sp = j == K * K - 1
                    for h in range(2):
                        base = h * HALF
                        for lo, n in ((0, 512), (512, HALF - 512)):
                            nc.tensor.matmul(
                                den[h][:, lo : lo + n],
                                ident[:],
                                e_t[:, base + lo : base + lo + n],
                                start=st,
                                stop=sp,
                            )
                            nc.tensor.matmul(
                                num[h][:, lo : lo + n],
                                ident[:],
                                w_t[:, base + lo : base + lo + n],
                                start=st,
                                stop=sp,
                            )

                # ---- epilogue ----
                out_sb = epool.tile([128, OP], F32, tag="osb", name=f"osb{c}")
                for h in range(2):
                    base = h * HALF
                    rden = epool.tile([128, HALF], F32, tag="rden", name=f"rd{c}{h}")
                    nc.vector.reciprocal_approx_fast(rden[:], den[h][:])
                    nc.vector.tensor_tensor(
                        out_sb[:, base : base + HALF],
                        num[h][:],
                        rden[:],
                        mybir.AluOpType.mult,
                    )
                nc.sync.dma_start(out_d[c], out_sb[:])

    nc.compile()
    return nc


def _block_diag_weights(w):
    """w: (G, Cg_out, Cg_in) -> lhsT layout [NCH, 128, 128] where
    lhsT[c, ci, co] = w[g, co%32, ci%32] for matching 32-blocks."""
    out = np.zeros((NCH, 128, 128), np.float32)
    for c in range(NCH):
        for g4 in range(4):
            g = c * 4 + g4
            blk = w[g]  # (Cg_out, Cg_in)
            out[c, g4 * 32 : (g4 + 1) * 32, g4 * 32 : (g4 + 1) * 32] = blk.T
    return out


_NC_CACHE = {}


def _make_in_maps(inputs):
    x = np.asarray(inputs["x"], np.float32)
    wq = np.asarray(inputs["wq"], np.float32)
    wk = np.asarray(inputs["wk"], np.float32)
    wv = np.asarray(inputs["wv"], np.float32)
    rel_emb = np.asarray(inputs["rel_emb"], np.float32)
    q_emb = np.asarray(inputs["q_emb"], np.float32)

    wqb = _block_diag_weights(wq)
    wkb = _block_diag_weights(wk)
    wvb = _block_diag_weights(wv)
    relb = np.ascontiguousarray(
        rel_emb.reshape(G, Cg, K * K).reshape(NCH, 128, K * K)
    )
    qeb = np.ascontiguousarray(q_emb.reshape(NCH, 128, 1))
    idn = np.eye(128, dtype=ml_dtypes.bfloat16)

    xp = np.pad(x, ((0, 0), (P, P), (P, P), (0, 0)))  # (B, 60, 60, C)

    in_maps = []
    for core in range(NCORES):
        b, half = divmod(core, 2)
        sh = xp[b, HS * half : HS * half + MR]         # (32, 60, C)
        xt = np.ascontiguousarray(sh.reshape(SP, C).T).reshape(NCH, 128, SP)
        in_maps.append(
            {
                "xt": xt,
                "wqb": wqb,
                "wkb": wkb,
                "wvb": wvb,
                "relb": relb,
                "qeb": qeb,
                "idn": idn,
            }
        )
    return in_maps


def kernel(**inputs):
    in_maps = _make_in_maps(inputs)

    if "nc" not in _NC_CACHE:
        _NC_CACHE["nc"] = build_nc()
    nc = _NC_CACHE["nc"]

    res = run_bass_kernel_spmd(nc, in_maps, core_ids=list(range(NCORES)))

    out = np.empty((B, H, W, C), np.float32)
    for core in range(NCORES):
        b, half = divmod(core, 2)
        o = res.results[core]["out"].reshape(C, HS, W)
        out[b, HS * half : HS * half + HS] = o.transpose(1, 2, 0)
    return out


# revision 10
# speedup vs baseline: 6.2392x; 6.2392x over previous
"""Trainium2 Bass kernel for AttentionConvFull (local 5x5 window attention
with per-channel softmax, grouped 1x1 conv projections).

Sharding: 8 cores = batch(4) x H-halves(2). Each core gets a 32-row halo'd,
zero-padded slice of x, pre-transposed on host to channel-major [256, 32*60].
No collectives needed.

Per-core dataflow (2 channel-chunks of 128 partitions each):
  PE    : block-diag 128x128 fp32 matmuls for q/k/v projections; per window
          offset j, identity-matmul PSUM accumulation of den += e_j and
          num += (e_j * v_j)  (bf16 inputs, fp32 accumulate)
  DMA   : creates 5 column-shifted flat copies of the k/v maps (SBUF->SBUF)
          so every j-loop operand is a flat, 4B-aligned [128, N] slice
  GpSimd: kr = k_dj + rel_j (tensor_scalar, per-partition scalar)
  DVE   : t = kr * q, w = e * v_dj (flat bf16 tensor_tensor, 2x mode)
  ACT   : e = exp(t); projection PSUM->SBUF casts (+q_emb bias for q)
  Epilogue: out = num * recip_approx(den), DMA out channel-major; host
  reassembles to (B,H,W,C).
"""

import numpy as np
import ml_dtypes

import concourse.bass as bass
import concourse.tile as tile
from concourse import bacc, mybir
from concourse.bass_utils import run_bass_kernel_spmd

F32 = mybir.dt.float32
BF16 = mybir.dt.bfloat16

K = 5
G = 8
B, H, W, C = 4, 56, 56, 256
Cg = C // G            # 32
P = K // 2             # 2
HS = H // 2            # 28 output rows per shard
MR = HS + 2 * P        # 32 map rows
MC = W + 2 * P         # 60 map cols
SP = MR * MC           # 1920 map spatial
OP = HS * W            # 1568 output spatial per shard
FM = MR * W            # 1792 flat shifted-map size
NCH = 2                # channel chunks of 128 partitions
NCORES = 8
HALF = OP // 2         # 784: PSUM accumulate tile half-size

# which engine does the kr = k + rel_j add: "gpsimd" or "vector"
# (gpsimd tensor_scalar measured 23us/op on HW + port-sharing serialization
#  against DVE -- unusable; keep on DVE where it hits the 4x perf mode)
RADD_ENGINE = "vector"


def _dedup_ldweights(nc):
    """Remove redundant PE weight reloads: consecutive InstLdweights that
    load the same stationary operand (same tensor + AP) with no sync info.
    The identity matrix stays resident across the whole accumulation loop,
    so only the first load is needed."""
    removed = 0
    for blk in nc.main_func.blocks:
        last_sig = None
        keep = []
        for inst in blk.instructions:
            if isinstance(inst, mybir.InstLdweights):
                sig = " ".join(a.concise() for a in inst.ins)
                si = inst.sync_info
                clean = si is None or (
                    len(si.on_wait) == 0 and len(si.on_update) == 0
                )
                if sig == last_sig and clean:
                    removed += 1
                    continue
                last_sig = sig
            elif isinstance(inst, mybir.InstMatmult):
                # a self-loading matmul (e.g. the fp32 projections) replaces
                # the stationary operand in the PE array
                if len(inst.ins) > 1:
                    wsig = inst.ins[1].concise()
                    if wsig != last_sig:
                        last_sig = wsig
            keep.append(inst)
        blk.instructions[:] = keep
    return removed


def build_nc():
    nc = bacc.Bacc(
        "TRN2", target_bir_lowering=False, debug=False, num_devices=NCORES
    )

    xt_d = nc.dram_tensor("xt", [NCH, 128, SP], F32, kind="ExternalInput").ap()
    wq_d = nc.dram_tensor("wqb", [NCH, 128, 128], F32, kind="ExternalInput").ap()
    wk_d = nc.dram_tensor("wkb", [NCH, 128, 128], F32, kind="ExternalInput").ap()
    wv_d = nc.dram_tensor("wvb", [NCH, 128, 128], F32, kind="ExternalInput").ap()
    rel_d = nc.dram_tensor("relb", [NCH, 128, K * K], F32, kind="ExternalInput").ap()
    qe_d = nc.dram_tensor("qeb", [NCH, 128, 1], F32, kind="ExternalInput").ap()
    id_d = nc.dram_tensor("idn", [128, 128], BF16, kind="ExternalInput").ap()
    out_d = nc.dram_tensor("out", [NCH, 128, OP], F32, kind="ExternalOutput").ap()

    with tile.TileContext(nc) as tc:
        with (
            tc.tile_pool(name="consts", bufs=1) as consts,
            tc.tile_pool(name="weights", bufs=2) as wpool,
            tc.tile_pool(name="xin", bufs=2) as xpool,
            tc.tile_pool(name="maps", bufs=2) as mpool,
            tc.tile_pool(name="jwork", bufs=4) as jpool,
            tc.tile_pool(name="epi", bufs=2) as epool,
            tc.tile_pool(name="acc", bufs=4, space=bass.MemorySpace.PSUM) as psum,
        ):
            ident = consts.tile([128, 128], BF16, tag="ident")
            nc.sync.dma_start(ident[:], id_d)

            # ---- per-chunk persistent maps ----
            kvars, vvars, qflats, rels = [], [], [], []

            for c in range(NCH):
                x_sb = xpool.tile([128, SP], F32, tag="x")
                nc.sync.dma_start(x_sb[:], xt_d[c])

                wts = {}
                for nm, d in (("wq", wq_d), ("wk", wk_d), ("wv", wv_d)):
                    t = wpool.tile([128, 128], F32, tag=nm)
                    nc.sync.dma_start(t[:], d[c])
                    wts[nm] = t
                rel_sb = wpool.tile([128, K * K], F32, tag="rel")
                nc.sync.dma_start(rel_sb[:], rel_d[c])
                qe_sb = wpool.tile([128, 1], F32, tag="qe")
                nc.sync.dma_start(qe_sb[:], qe_d[c])
                rels.append(rel_sb)

                # base maps (bf16), psum -> sbuf casts on ACT
                k_bf = mpool.tile([128, SP], BF16, tag="k")
                v_bf = mpool.tile([128, SP], BF16, tag="v")
                q_bf = mpool.tile([128, SP], BF16, tag="q")

                # projections: 4 slices of 480 cols each
                NS = 4
                SL = SP // NS  # 480
                for s in range(NS):
                    lo = s * SL
                    rhs = x_sb[:, lo : lo + SL]
                    for nm in ("wk", "wv", "wq"):
                        ps = psum.tile([128, HALF], F32, tag="acc", name=f"pp{c}{s}{nm}")
                        nc.tensor.matmul(
                            ps[:, :SL], wts[nm][:], rhs, start=True, stop=True
                        )
                        if nm == "wq":
                            # q = proj + q_emb (per-partition bias), cast bf16
                            nc.scalar.activation(
                                q_bf[:, lo : lo + SL],
                                ps[:, :SL],
                                mybir.ActivationFunctionType.Identity,
                                bias=qe_sb[:],
                            )
                        elif nm == "wk":
                            nc.scalar.copy(k_bf[:, lo : lo + SL], ps[:, :SL])
                        else:
                            nc.scalar.copy(v_bf[:, lo : lo + SL], ps[:, :SL])

                # flat dj-shifted variants via SBUF->SBUF DMA (free engines):
                # k_dj[h*56+w] = k[h, w+dj], shape [128, 32*56]
                k3 = k_bf[:].rearrange("p (h w) -> p h w", h=MR)
                v3 = v_bf[:].rearrange("p (h w) -> p h w", h=MR)
                kvs, vvs = [], []
                for dj in range(K):
                    kd = mpool.tile([128, FM], BF16, tag=f"kd{dj}", name=f"kd{c}{dj}")
                    nc.sync.dma_start(
                        kd[:].rearrange("p (h w) -> p h w", h=MR),
                        k3[:, :, dj : dj + W],
                    )
                    kvs.append(kd)
                    vd = mpool.tile([128, FM], BF16, tag=f"vd{dj}", name=f"vd{c}{dj}")
                    nc.sync.dma_start(
                        vd[:].rearrange("p (h w) -> p h w", h=MR),
                        v3[:, :, dj : dj + W],
                    )
                    vvs.append(vd)
                kvars.append(kvs)
                vvars.append(vvs)

                qf = mpool.tile([128, OP], BF16, tag="qf", name=f"qf{c}")
                q3 = q_bf[:].rearrange("p (h w) -> p h w", h=MR)
                nc.sync.dma_start(
                    qf[:].rearrange("p (h w) -> p h w", h=HS),
                    q3[:, P : P + HS, P : P + W],
                )
                qflats.append(qf)

            # ---- j-loop per chunk ----
            for c in range(NCH):
                rel_sb, qf = rels[c], qflats[c]

                den = [
                    psum.tile([128, HALF], F32, tag="acc", name=f"den{c}{h}")
                    for h in range(2)
                ]
                num = [
                    psum.tile([128, HALF], F32, tag="acc", name=f"num{c}{h}")
                    for h in range(2)
                ]

                for j in range(K * K):
                    di, dj = divmod(j, K)
                    o = di * W
                    kv = kvars[c][dj][:, o : o + OP]
                    vv = vvars[c][dj][:, o : o + OP]

                    kr_t = jpool.tile([128, OP], BF16, tag="kr", name=f"kr{c}{j}")
                    radd = nc.gpsimd if RADD_ENGINE == "gpsimd" else nc.vector
                    radd.tensor_scalar(
                        kr_t[:],
                        kv,
                        rel_sb[:, j : j + 1],
                        None,
                        mybir.AluOpType.add,
                    )

                    t_t = jpool.tile([128, OP], BF16, tag="t", name=f"t{c}{j}")
                    nc.vector.tensor_tensor(
                        t_t[:], kr_t[:], qf[:], mybir.AluOpType.mult
                    )

                    e_t = jpool.tile([128, OP], BF16, tag="e", name=f"e{c}{j}")
                    nc.scalar.activation(
                        e_t[:], t_t[:], mybir.ActivationFunctionType.Exp
                    )

                    w_t = jpool.tile([128, OP], BF16, tag="w", name=f"w{c}{j}")
                    nc.vector.tensor_tensor(
                        w_t[:], e_t[:], vv, mybir.AluOpType.mult
                    )

                    st = j == 0
                    sp = j == K * K - 1
                    for h in range(2):
                        base = h * HALF
                        for lo, n in ((0, 512), (512, HALF - 512)):
                            nc.tensor.matmul(
                                den[h][:, lo : lo + n],
                                ident[:],
                                e_t[:, base + lo : base + lo + n],
                                start=st,
                                stop=sp,
                            )
                            nc.tensor.matmul(
                                num[h][:, lo : lo + n],
                                ident[:],
                                w_t[:, base + lo : base + lo + n],
                                start=st,
                                stop=sp,
                            )

                # ---- epilogue ----
                out_sb = epool.tile([128, OP], F32, tag="osb", name=f"osb{c}")
                for h in range(2):
                    base = h * HALF
                    rden = epool.tile([128, HALF], F32, tag="rden", name=f"rd{c}{h}")
                    nc.vector.reciprocal_approx_fast(rden[:], den[h][:])
                    nc.vector.tensor_tensor(
                        out_sb[:, base : base + HALF],
                        num[h][:],
                        rden[:],
                        mybir.AluOpType.mult,
                    )
                nc.sync.dma_start(out_d[c], out_sb[:])

    nc.compile()
    _dedup_ldweights(nc)
    return nc


def _block_diag_weights(w):
    """w: (G, Cg_out, Cg_in) -> lhsT layout [NCH, 128, 128] where
    lhsT[c, ci, co] = w[g, co%32, ci%32] for matching 32-blocks."""
    out = np.zeros((NCH, 128, 128), np.float32)
    for c in range(NCH):
        for g4 in range(4):
            g = c * 4 + g4
            blk = w[g]  # (Cg_out, Cg_in)
            out[c, g4 * 32 : (g4 + 1) * 32, g4 * 32 : (g4 + 1) * 32] = blk.T
    return out


_NC_CACHE = {}


def _make_in_maps(inputs):
    x = np.asarray(inputs["x"], np.float32)
    wq = np.asarray(inputs["wq"], np.float32)
    wk = np.asarray(inputs["wk"], np.float32)
    wv = np.asarray(inputs["wv"], np.float32)
    rel_emb = np.asarray(inputs["rel_emb"], np.float32)
    q_emb = np.asarray(inputs["q_emb"], np.float32)

    wqb = _block_diag_weights(wq)
    wkb = _block_diag_weights(wk)
    wvb = _block_diag_weights(wv)
    relb = np.ascontiguousarray(
        rel_emb.reshape(G, Cg, K * K).reshape(NCH, 128, K * K)
    )
    qeb = np.ascontiguousarray(q_emb.reshape(NCH, 128, 1))
    idn = np.eye(128, dtype=ml_dtypes.bfloat16)

    xp = np.pad(x, ((0, 0), (P, P), (P, P), (0, 0)))  # (B, 60, 60, C)

    in_maps = []
    for core in range(NCORES):
        b, half = divmod(core, 2)
        sh = xp[b, HS * half : HS * half + MR]         # (32, 60, C)
        xt = np.ascontiguousarray(sh.reshape(SP, C).T).reshape(NCH, 128, SP)
        in_maps.append(
            {
                "xt": xt,
                "wqb": wqb,
                "wkb": wkb,
                "wvb": wvb,
                "relb": relb,
                "qeb": qeb,
                "idn": idn,
            }
        )
    return in_maps


def kernel(**inputs):
    in_maps = _make_in_maps(inputs)

    if "nc" not in _NC_CACHE:
        _NC_CACHE["nc"] = build_nc()
    nc = _NC_CACHE["nc"]

    res = run_bass_kernel_spmd(nc, in_maps, core_ids=list(range(NCORES)))

    out = np.empty((B, H, W, C), np.float32)
    for core in range(NCORES):
        b, half = divmod(core, 2)
        o = res.results[core]["out"].reshape(C, HS, W)
        out[b, HS * half : HS * half + HS] = o.transpose(1, 2, 0)
    return out
